# revision 1
# baseline (speedup 1.0000x reference)
"""AdaXbmTripletLoss kernel for 8 Trainium2 NeuronCores (Bass/Tile).

Reference math: loss = sum(hard * relu(d_ap + sqrt(margin) - d_an)) / count(hard)
with hard = ~is_nonneg & (sim > pos_sim - margin) & has_q, over [B=256, M=32768].

Device strategy (inputs_row sharded 8 ways -> ML=4096 rows/core; queries
replicated; all [B, M] work on-device in m-tiles of GM=2048):

z-space trick: host scales each query by 1/delta_b^2 (delta_b = the
d_an threshold sqrt(alpha - 2*thr)), so on device
    z = bias_b - psum/128 = d_an^2 / delta_b^2
and the mask compare becomes the GLOBAL constant 1.0:
    masked  <=>  z < 1  <=>  sqrt(z) < 1.
That removes every per-partition operand from the DVE epilogue ops, so
tensor_scalar runs with immediate scalars on bf16 data -> 4x DVE mode.

Per (g, bt) tile [128 queries x 2048 m]:
  PE:  8 matmuls fp8 DoubleRow -> psum f32 (= 256*sim/delta^2)
  ACT: sqz = Sqrt(-psum/128 + bias_b)  -> bf16 SBUF  [drains PSUM; the
       only per-element ACT pass - ACT is 1 elem/cycle/lane, ~2.4us/tile]
  DVE: count C  = accum is_lt(sqz, 1.0)      [4x mode, immediate scalar]
       Smin  S  = accum min(sqz, 1.0)        [4x mode, immediate scalar]
Host per tile: smask += delta*(S - (n - C)); total_b = gamma*count_b - smask_b.
Identity is exact per element for whatever rounding the device applied
(C and S come from the same bf16 sqz values).

DMA: three parallel issue paths (SP HWDGE, ACT HWDGE, GpSimd SWDGE) so the
row halves stream concurrently instead of FIFO-serializing on one ring.
Dummy ldweights absorb the rows-DMA waits (1-embedded-wait walrus limit);
PE clock pre-warmed with dummy matmuls during the DMA window.

Host (numpy, microseconds): index preprocessing, per-query constants in
f64, reduction of the [128, 8] per-core outputs, the sparse is_nonneg
correction (~900 of 8.4M pairs, exact f64), and exact fallbacks for
delta > gamma rows or non-finite device output (never trigger here).
"""

import os
import numpy as np
import ml_dtypes

B = 256
NCOL = 512
M = 32768
D = 512
K = 10
MARGIN = 0.1
EPS = 1e-6
TMARGIN = MARGIN ** 0.5
NCORES = 8
ML = M // NCORES          # 4096 rows per core
DCH = D // 128            # 4 contraction chunks
BT = B // 128             # 2 b-tiles
GM = 2048                 # m-tile size
G = ML // GM              # 2 groups
HM = GM // 2              # rows DMA half size

_cache = {}
last_run = {}             # exec_time_ns etc. for test harness introspection


def _patch_tile_drain():
    """This container's walrus build allows only ONE embedded sync wait per
    instruction, but TileContext's kernel-tail drain aggregates a wait per
    logical proc (engines + DMA queues) onto a single Drain instruction ->
    'Too many sync wait commands'.  Replace it with standalone single-wait
    wait_ge instructions on the sync engine followed by a bare drain."""
    import concourse.tile as tile
    from concourse.tile_sem_assignment import tick_to_sem

    if getattr(tile.TileContext, "_drain_patched", False):
        return

    def _drain_and_barrier(self, tick_clock, wait_clock):
        gc = tick_clock.global_clock
        assert self.sems is not None
        for proc_idx, sem in sorted(self.sems.allocated().items()):
            tick = gc[proc_idx]
            if tick > 0:
                self.nc.sync.wait_ge(sem, tick_to_sem(tick, proc_idx))
        self.nc.sync.drain()
        self.nc.all_engine_barrier()
        popped = self.nc._tile_sem_poison_stack.pop()
        assert popped is self._sem_poison
        self.nc.clear_and_free_semaphores(list(self.sems.allocated().values()))
        self.nc.all_engine_barrier()

    tile.TileContext._drain_and_barrier = _drain_and_barrier
    tile.TileContext._drain_patched = True


def _build_nc():
    import concourse.bass as bass
    import concourse.mybir as mybir
    import concourse.tile as tile

    _patch_tile_drain()
    nc = bass.Bass()
    f32 = mybir.dt.float32
    bf16 = mybir.dt.bfloat16
    fp8 = mybir.dt.float8e4

    # rows: per (group, half) [128, DCH, HM] fp8; each chunk is DMA'd as two
    # DCH-halves so the SP and ACT HWDGE rings stream it in parallel, and the
    # ring queues are ordered by PE consumption order (the SDMA engines
    # round-robin the queues, so queue order == global delivery order).
    rows_ext = [
        [nc.declare_dram_parameter(f"rows{g}{h}", [128, DCH, HM], fp8, False)
         for h in range(2)]
        for g in range(G)
    ]
    q_ext = nc.declare_dram_parameter("q_t", [128, DCH, B], fp8, False)
    # consts columns: bias (= alpha/delta^2) for bt0, bt1; ones for Sign bias
    consts_ext = nc.declare_dram_parameter("consts", [128, 4], f32, False)
    # out: [0:2] C of t0,t1 (DVE reduce), [2:6] S of t0..t3 (DVE reduce),
    #      [6:8] sign-accums of t2,t3 (ACT) -> C = (acc + GM)/2
    out_ext = nc.declare_dram_parameter("out", [128, 8], f32, True)

    with tile.TileContext(nc) as tc:
        with (
            tc.tile_pool(name="rows", bufs=1) as rows_pool,
            tc.tile_pool(name="qt", bufs=1) as qt_pool,
            tc.tile_pool(name="consts", bufs=1) as consts_pool,
            tc.tile_pool(name="psum", bufs=2, space="PSUM") as psum_pool,
            tc.tile_pool(name="sqz", bufs=4) as sqz_pool,
            tc.tile_pool(name="scr", bufs=3) as scr_pool,
            tc.tile_pool(name="cols", bufs=1) as cols_pool,
        ):
            rows_tiles = [
                [rows_pool.tile([128, DCH, HM], fp8, tag=f"rows{g}{h}",
                                name=f"rows{g}{h}") for h in range(2)]
                for g in range(G)
            ]
            qt_tile = qt_pool.tile([128, DCH, B], fp8)
            consts_tile = consts_pool.tile([128, 4], f32)

            # DMA plan: SP ring gets the low-DCH half of every rows chunk,
            # ACT ring the high-DCH half, both in consumption order; the
            # gpsimd SWDGE ring carries consts+q.  Each chunk lands at
            # ~full aggregate bandwidth as the engines drain queue heads.
            for g in range(G):
                for h in range(2):
                    nc.sync.dma_start(rows_tiles[g][h][:, 0:2], rows_ext[g][h][:, 0:2])
            for g in range(G):
                for h in range(2):
                    nc.scalar.dma_start(rows_tiles[g][h][:, 2:4], rows_ext[g][h][:, 2:4])
            nc.gpsimd.dma_start(consts_tile[:], consts_ext[:])
            nc.gpsimd.dma_start(qt_tile[:], q_ext[:])

            # Warm-up: ACT sqrt on a consts column pulls the Sqrt table load
            # off the critical path and absorbs the consts-DMA wait before
            # the first real sqrt (which already carries its PE wait).
            warm = consts_pool.tile([128, 1], f32)
            nc.scalar.activation(
                warm[:], consts_tile[:, 0:1],
                mybir.ActivationFunctionType.Sqrt,
            )

            # PE clock warm-up: HAM runs the PE at 1.2GHz until ~4us of
            # sustained activity.  Dummy matmuls on scratch data while the
            # rows DMAs are in flight get the real matmuls to ~2.4GHz.
            wsrc = consts_pool.tile([128, 128], bf16)
            nc.gpsimd.memset(wsrc[:], 0.0)
            pwarm = psum_pool.tile([128, 512], f32, tag="psum", name="pwarm")
            for _ in range(7):
                nc.tensor.matmul(pwarm[:], wsrc[:], wsrc[:, 0:1].broadcast_to((128, 512)))

            cols = cols_pool.tile([128, 8], f32)
            CCOL = {0: 0, 1: 1}          # DVE-reduced counts
            SCOL = {0: 2, 1: 3, 2: 4, 3: 5}
            ACOL = {2: 6, 3: 7}          # ACT sign-accum counts
            ones_ap = consts_tile[:, 2:3]
            sign_jobs = []

            for g in range(G):
                for bt in range(BT):
                    t = 2 * g + bt
                    bias_ap = consts_tile[:, bt : bt + 1]
                    psum = psum_pool.tile([128, GM], f32, tag="psum",
                                          name=f"ps{g}_{bt}")
                    # h-major so the first 4 matmuls only need the low m-half
                    # of the group; dummy ldweights absorb each rows-half's
                    # two DMA waits right before its first use (bt==0 only)
                    for h in range(GM // 512):
                        if bt == 0 and h % 2 == 0:
                            rh = rows_tiles[g][h // 2]
                            nc.tensor.ldweights(rh[:, 0, 0:1])
                            nc.tensor.ldweights(rh[:, 2, 0:1])
                        hsl = slice(h * 512, (h + 1) * 512)
                        for dp in range(DCH // 2):
                            lhs = qt_tile[:, 2 * dp : 2 * dp + 2,
                                          bt * 128 : (bt + 1) * 128]
                            rhs = rows_tiles[g][h // 2][
                                :, 2 * dp : 2 * dp + 2,
                                (h % 2) * 512 : (h % 2) * 512 + 512]
                            nc.tensor.matmul(
                                psum[:, hsl],
                                lhs,
                                rhs,
                                start=(dp == 0),
                                stop=(dp == DCH // 2 - 1),
                                perf_mode=mybir.MatmulPerfMode.DoubleRow,
                            )
                    # sqz = sqrt(bias - psum/128) = d_an/delta, in bf16 so
                    # the DVE epilogue ops hit 4x / 2x modes
                    sqz = sqz_pool.tile([128, GM], bf16, tag="sqz",
                                        name=f"sqz{g}_{bt}")
                    nc.scalar.activation(
                        sqz[:], psum[:], mybir.ActivationFunctionType.Sqrt,
                        bias=bias_ap, scale=-2.0 / 256.0,
                    )
                    # count C = sum 1[sqz < 1]: DVE 4x transform + 2x reduce
                    # for t0,t1; ACT Sign+accum for t2,t3 (load balance)
                    if t in CCOL:
                        ind = scr_pool.tile([128, GM], bf16, tag="scr",
                                            name=f"ind{t}")
                        nc.vector.tensor_scalar(
                            ind[:], sqz[:], 1.0, None,
                            op0=mybir.AluOpType.is_lt,
                        )
                        nc.vector.tensor_reduce(
                            cols[:, CCOL[t] : CCOL[t] + 1], ind[:],
                            axis=mybir.AxisListType.X, op=mybir.AluOpType.add,
                        )
                    else:
                        sign_jobs.append((t, sqz))
                    # S = sum min(sqz, 1) = sum_masked sqz + (n - C)
                    mn = scr_pool.tile([128, GM], bf16, tag="scr",
                                       name=f"mn{t}")
                    nc.vector.tensor_scalar(
                        mn[:], sqz[:], 1.0, None,
                        op0=mybir.AluOpType.min,
                    )
                    nc.vector.tensor_reduce(
                        cols[:, SCOL[t] : SCOL[t] + 1], mn[:],
                        axis=mybir.AxisListType.X, op=mybir.AluOpType.add,
                    )

            # deferred ACT count jobs run after the last sqrt
            for t, sqz in sign_jobs:
                sg = scr_pool.tile([128, GM], bf16, tag="scr", name=f"sg{t}")
                nc.scalar.activation(
                    sg[:], sqz[:], mybir.ActivationFunctionType.Sign,
                    bias=ones_ap, scale=-1.0,
                    accum_out=cols[:, ACOL[t] : ACOL[t] + 1],
                )

            # split out DMA by writer engine so each carries one sync wait;
            # both ride the SWDGE ring whose completion-sem lanes are not
            # recycled (the 8 HWDGE lanes are all taken by the rows DMAs)
            nc.gpsimd.dma_start(out_ext[:, 0:6], cols[:, 0:6])
            nc.gpsimd.dma_start(out_ext[:, 6:8], cols[:, 6:8])

    # Post-pass: the walrus build allows one embedded sync wait per
    # instruction, but Tile emits conservative same-engine self-waits (e.g.
    # a matmul's PE wait when evicting a PSUM slot, or a DVE op's DVE wait
    # when recycling a scratch buffer).  An engine executes its own queue in
    # order, so a wait on the engine's own earlier tick is always implied -
    # drop self-waits whenever another wait remains.
    _eng_prefix = {
        "PE": "PE_", "Activation": "Activation_", "DVE": "DVE_",
        "Pool": "Pool_", "SP": "SP_",
    }
    for bb in nc.m.functions[0].blocks:
        for i in bb.instructions:
            si = i.sync_info
            if si is None:
                continue
            w = si.on_wait
            if len(w) < 2:
                continue
            pref = _eng_prefix.get(getattr(i.engine, "name", None) or str(i.engine))
            if pref is None:
                continue
            keep = [x for x in w if not x.ant_name.startswith(pref)]
            if 1 <= len(keep) < len(w):
                si.on_wait = keep

    return nc


def _get_nc():
    if "nc" not in _cache:
        _cache["nc"] = _build_nc()
    return _cache["nc"]


def _install_ntff_hook():
    """The agent image's antenv lacks axon_hooks; shim it from trn_agent_boot so
    run_bass_kernel_spmd(trace=True) can capture NTFF profiles under axon."""
    import sys
    import types
    try:
        import antenv.axon_hooks  # noqa: F401
        return
    except ImportError:
        pass
    try:
        import antenv
        from trn_agent_boot.trn_boot import _ntff_profile_via_ctypes
        hook = {"h": _ntff_profile_via_ctypes("/opt/axon/libaxon_pjrt.so")}
        mod = types.ModuleType("antenv.axon_hooks")
        mod.get_axon_ntff_profile_hook = lambda: hook["h"]
        mod.set_axon_ntff_profile_hook = lambda h: hook.__setitem__("h", h)
        sys.modules["antenv.axon_hooks"] = mod
        antenv.axon_hooks = mod
    except Exception:
        pass


def kernel(inputs_col, inputs_row, targets_col, targets_row, qidxs, pidxs, nnegs, bs):
    from concourse.bass_utils import run_bass_kernel_spmd

    bs = int(np.asarray(bs))
    assert bs == B and inputs_row.shape == (M, D) and inputs_col.shape[1] == D

    inputs_col = np.asarray(inputs_col, dtype=np.float32)
    inputs_row = np.asarray(inputs_row, dtype=np.float32)
    targets_col = np.asarray(targets_col)
    targets_row = np.asarray(targets_row)
    qidxs = np.asarray(qidxs)
    nnegs = np.asarray(nnegs)

    q = inputs_col[:bs]                                        # [B, D] f32

    # ---- host-side index preprocessing (tiny int ops) ----
    match = targets_col[:bs, None] == qidxs[None, :]
    has_q = match.any(axis=1)
    qloc = match.argmax(axis=1)
    my_nnegs = nnegs[qloc]                                     # [B, K]

    pos_idx = bs + np.arange(bs)
    p = inputs_row[pos_idx]                                    # [B, D] f32

    # ---- per-query constants (f64 host math) ----
    q64 = q.astype(np.float64)
    p64 = p.astype(np.float64)
    na = (q64 * q64).sum(1)
    sa = q64.sum(1)
    # device z = (alpha - 2*sim)/delta^2 with beta_m = |r_m|^2 - 2*eps*sum(r_m)
    # ~= 1 folded in (rows are L2-normalized), so alpha includes the +1.
    alpha = na + 2.0 * EPS * sa + D * EPS * EPS + 1.0
    d_ap = np.sqrt(((q64 - p64 + EPS) ** 2).sum(1))
    gamma = d_ap + TMARGIN
    pos_sim = (q64 * p64).sum(1)
    thr = pos_sim - MARGIN
    delta2 = alpha - 2.0 * thr                 # >= 0.2 (alpha ~ 2, pos_sim <= 1)
    delta = np.where(has_q, np.sqrt(np.maximum(delta2, 1e-12)), 0.0)
    s2 = np.where(has_q, 1.0 / delta2, 0.0)
    bias = np.where(has_q, alpha * s2, 2.0)
    # rows where the masked-sum identity breaks -> exact host fallback
    bad_b = np.flatnonzero(has_q & (delta > gamma))

    # ---- device inputs ----
    # rows{g}{h} per core: [128, DCH, HM], rows[p, k, m] =
    #   inputs_row[c*ML + g*GM + h*HM + m, k*128 + p] * 16 in fp8
    rt = (inputs_row.T * np.float32(16.0)).astype(ml_dtypes.float8_e4m3)  # [D, M]
    rt = rt.reshape(DCH, 128, NCORES, G, 2, HM)             # k, p, c, g, h, m
    qp = (q64 * (16.0 * s2[:, None])).astype(np.float32)
    q_t = qp.T.astype(ml_dtypes.float8_e4m3).reshape(DCH, 128, B)
    q_t = np.ascontiguousarray(q_t.transpose(1, 0, 2))      # [128, DCH, B]
    consts = np.empty((128, 4), np.float32)
    consts[:, 0] = bias[:128]
    consts[:, 1] = bias[128:]
    consts[:, 2] = 1.0
    consts[:, 3] = 0.0

    in_maps = []
    for c in range(NCORES):
        rc = rt[:, :, c].transpose(2, 3, 1, 0, 4)           # [G, 2, 128, DCH, HM]
        m = {"q_t": q_t, "consts": consts}
        for g in range(G):
            for h in range(2):
                m[f"rows{g}{h}"] = np.ascontiguousarray(rc[g, h])
        in_maps.append(m)

    nc = _get_nc()
    trace = bool(os.environ.get("ATHENA_KERNEL_TRACE"))
    if trace:
        _install_ntff_hook()
    r = run_bass_kernel_spmd(nc, in_maps, list(range(NCORES)), trace=trace)
    last_run["exec_time_ns"] = r.exec_time_ns
    last_run["results"] = r

    # ---- gather partials ----
    # cols: [0:2] C of t0,t1; [2:6] S of t0..t3; [6:8] sign-accum of t2,t3
    count_b = np.zeros(B, np.float64)
    smask_b = np.zeros(B, np.float64)   # sum over masked of d_an
    for c in range(NCORES):
        o = np.asarray(r.results[c]["out"], dtype=np.float64)  # [128, 8]
        for g in range(G):
            for bt in range(BT):
                t = 2 * g + bt
                sl = slice(bt * 128, (bt + 1) * 128)
                if t < 2:
                    C = o[:, t]
                else:
                    C = (o[:, 4 + t] + GM) / 2.0
                S = o[:, 2 + t]
                count_b[sl] += C
                # sum_masked d_an = delta * (S - (n - C))
                smask_b[sl] += delta[sl] * (S - (GM - C))
    total_b = gamma * count_b - smask_b

    # ---- exact host fallback for identity violations / non-finite output ----
    bad = set(int(b) for b in bad_b)
    nf = np.flatnonzero(~(np.isfinite(total_b) & np.isfinite(count_b)))
    bad.update(int(b) for b in nf if has_q[b])
    for b in nf:
        if not has_q[b]:
            count_b[b] = 0.0
            total_b[b] = 0.0
    if bad:
        rows64 = inputs_row.astype(np.float64)
        nb_all = (rows64 * rows64).sum(1)
        sb_all = rows64.sum(1)
        for b in sorted(bad):
            simrow = rows64 @ q64[b]
            mask = simrow > thr[b]
            d2 = (na[b] + nb_all - 2.0 * simrow
                  + 2.0 * EPS * (sa[b] - sb_all) + D * EPS * EPS)
            d_an = np.sqrt(np.maximum(d2, 0.0))
            count_b[b] = mask.sum()
            total_b[b] = np.maximum(gamma[b] - d_an, 0.0)[mask].sum()

    # ---- sparse is_nonneg correction (host, exact) ----
    order = np.argsort(targets_row, kind="stable")
    tr_sorted = targets_row[order]
    lo = np.searchsorted(tr_sorted, my_nnegs.ravel(), side="left")
    hi = np.searchsorted(tr_sorted, my_nnegs.ravel(), side="right")
    pairs = set()
    for flat, (l, h) in enumerate(zip(lo, hi)):
        if h > l:
            b = flat // K
            if has_q[b]:
                for mm_ in order[l:h]:
                    pairs.add((b, int(mm_)))
    if pairs:
        pb = np.fromiter((x[0] for x in pairs), np.int64, len(pairs))
        pm = np.fromiter((x[1] for x in pairs), np.int64, len(pairs))
        rows_sel = inputs_row[pm].astype(np.float64)
        sims = (q64[pb] * rows_sel).sum(1)
        sel = sims > thr[pb]
        pb, pm, sims, rows_sel = pb[sel], pm[sel], sims[sel], rows_sel[sel]
        nb = (rows_sel * rows_sel).sum(1)
        sb = rows_sel.sum(1)
        d2 = na[pb] + nb - 2.0 * sims + 2.0 * EPS * (sa[pb] - sb) + D * EPS * EPS
        d_an = np.sqrt(np.maximum(d2, 0.0))
        tl = np.maximum(gamma[pb] - d_an, 0.0)
        np.add.at(count_b, pb, -1.0)
        np.add.at(total_b, pb, -tl)

    neg_count = count_b.sum()
    total = total_b.sum()
    loss = total / neg_count if neg_count > 0 else 0.0
    return np.float32(loss)



# revision 11
# speedup vs baseline: 1.0961x; 1.0961x over previous
"""AdaXbmTripletLoss kernel for 8 Trainium2 NeuronCores (Bass/Tile).

Reference math: loss = sum(hard * relu(d_ap + sqrt(margin) - d_an)) / count(hard)
with hard = ~is_nonneg & (sim > pos_sim - margin) & has_q, over [B=256, M=32768].

Device strategy (inputs_row sharded 8 ways -> ML=4096 rows/core; queries
replicated; all [B, M] work on-device in m-tiles of MT=1024):

z-space trick: host scales each query by 1/delta_b^2 (delta_b = the
d_an threshold sqrt(alpha - 2*thr)), so on device
    z = bias_b - psum/128 = d_an^2 / delta_b^2
and the mask compare becomes the GLOBAL constant 1.0:
    masked  <=>  z < 1  <=>  sqrt(z) < 1.
That removes every per-partition operand from the DVE epilogue ops, so
tensor_scalar runs with immediate scalars on bf16 data -> 4x DVE mode.

Per (c, bt) tile [128 queries x 1024 m]:
  PE:  4 matmuls fp8 DoubleRow -> psum f32 (= 256*sim/delta^2)
  ACT: sqz = Sqrt(-psum/128 + bias_b)  -> bf16 SBUF  [drains PSUM]
  DVE: tensor_scalar is_lt 1.0 with accum_out -> C   [4x mode, fused reduce]
       tensor_scalar min   1.0 with accum_out -> S   [4x mode, fused reduce]
Host per tile: smask += delta*(S - (n - C)); total_b = gamma*count_b - smask_b.
Identity is exact per element for whatever rounding the device applied
(C and S come from the same bf16 sqz values).

DMA: rows chunks alternate DCH-halves on the SP and ACT HWDGE rings in
consumption order, with q's halves FIRST on each ring so the queries land
before the first rows chunk (they previously rode the slow SWDGE ring and
stalled PE by ~4us).  Dummy ldweights absorb the DMA waits
(1-embedded-wait walrus limit); PE clock pre-warmed with dummy matmuls
during the DMA window.  The [128,16] partials return via a single SP-ring
DMA carrying one DVE wait.

Host (numpy, microseconds): index preprocessing, per-query constants in
f64, reduction of the [128, 16] per-core outputs, the sparse is_nonneg
correction (~900 of 8.4M pairs, exact f64), and exact fallbacks for
delta > gamma rows or non-finite device output (never trigger here).
"""

import os
import numpy as np
import ml_dtypes

B = 256
NCOL = 512
M = 32768
D = 512
K = 10
MARGIN = 0.1
EPS = 1e-6
TMARGIN = MARGIN ** 0.5
NCORES = 8
ML = M // NCORES          # 4096 rows per core
DCH = D // 128            # 4 contraction chunks
BT = B // 128             # 2 b-tiles
MT = 1024                 # m-tile size == DMA chunk granularity
NC_CH = ML // MT          # 4 chunks per core
NWARM = 7                 # dummy matmuls to ramp the PE clock

_cache = {}
last_run = {}             # exec_time_ns etc. for test harness introspection


def _patch_tile_drain():
    """This container's walrus build allows only ONE embedded sync wait per
    instruction, but TileContext's kernel-tail drain aggregates a wait per
    logical proc (engines + DMA queues) onto a single Drain instruction ->
    'Too many sync wait commands'.  Replace it with standalone single-wait
    wait_ge instructions on the sync engine followed by a bare drain."""
    import concourse.tile as tile
    from concourse.tile_sem_assignment import tick_to_sem

    if getattr(tile.TileContext, "_drain_patched", False):
        return

    def _drain_and_barrier(self, tick_clock, wait_clock):
        gc = tick_clock.global_clock
        assert self.sems is not None
        for proc_idx, sem in sorted(self.sems.allocated().items()):
            tick = gc[proc_idx]
            if tick > 0:
                self.nc.sync.wait_ge(sem, tick_to_sem(tick, proc_idx))
        self.nc.sync.drain()
        self.nc.all_engine_barrier()
        popped = self.nc._tile_sem_poison_stack.pop()
        assert popped is self._sem_poison
        self.nc.clear_and_free_semaphores(list(self.sems.allocated().values()))
        self.nc.all_engine_barrier()

    tile.TileContext._drain_and_barrier = _drain_and_barrier
    tile.TileContext._drain_patched = True


def _build_nc():
    import concourse.bass as bass
    import concourse.mybir as mybir
    import concourse.tile as tile

    _patch_tile_drain()
    nc = bass.Bass()
    f32 = mybir.dt.float32
    bf16 = mybir.dt.bfloat16
    fp8 = mybir.dt.float8e4

    # rows: per chunk [128, DCH, MT] fp8; each chunk is DMA'd as two
    # DCH-halves so the SP and ACT HWDGE rings stream it in parallel, and the
    # ring queues are ordered by PE consumption order (the SDMA engines
    # round-robin the queues, so queue order == global delivery order).
    rows_ext = [nc.declare_dram_parameter(f"rows{c}", [128, DCH, MT], fp8, False)
                for c in range(NC_CH)]
    q_ext = nc.declare_dram_parameter("q_t", [128, DCH, B], fp8, False)
    # consts columns: bias (= alpha/delta^2) for bt0, bt1
    consts_ext = nc.declare_dram_parameter("consts", [128, 2], f32, False)
    # out: col 2t = C of tile t, col 2t+1 = S of tile t  (t = 2*c + bt)
    out_ext = nc.declare_dram_parameter("out", [128, 4 * NC_CH], f32, True)

    with tile.TileContext(nc) as tc:
        with (
            tc.tile_pool(name="rows", bufs=1) as rows_pool,
            tc.tile_pool(name="qt", bufs=1) as qt_pool,
            tc.tile_pool(name="consts", bufs=1) as consts_pool,
            tc.tile_pool(name="psum", bufs=4, space="PSUM") as psum_pool,
            tc.tile_pool(name="sqz", bufs=8) as sqz_pool,
            tc.tile_pool(name="scr", bufs=2) as scr_pool,
            tc.tile_pool(name="cols", bufs=1) as cols_pool,
        ):
            rows_tiles = [rows_pool.tile([128, DCH, MT], fp8, tag=f"rows{c}",
                                         name=f"rows{c}") for c in range(NC_CH)]
            qt_tile = qt_pool.tile([128, DCH, B], fp8)
            consts_tile = consts_pool.tile([128, 2], f32)

            # DMA plan: SP ring gets the low-DCH half of q then every rows
            # chunk, ACT ring the high-DCH halves, both in consumption order
            # (the SDMA engines round-robin the queues, so queue order ==
            # global delivery order); q first so the queries land before the
            # first rows chunk.  consts ride the SP ring last (only needed
            # by the first real sqrt, well after they arrive).
            nc.sync.dma_start(qt_tile[:, 0:2], q_ext[:, 0:2])
            nc.scalar.dma_start(qt_tile[:, 2:4], q_ext[:, 2:4])
            for c in range(NC_CH):
                nc.sync.dma_start(rows_tiles[c][:, 0:2], rows_ext[c][:, 0:2])
            for c in range(NC_CH):
                nc.scalar.dma_start(rows_tiles[c][:, 2:4], rows_ext[c][:, 2:4])
            # consts on the gpsimd SWDGE ring: keeps the HWDGE completion-sem
            # lane recycling down to benign rows-on-q waits (tiny transfer,
            # only needed by the first real sqrt)
            nc.gpsimd.dma_start(consts_tile[:], consts_ext[:])

            # PE clock warm-up: HAM runs the PE at low clock until ~3us of
            # sustained activity.  Dummy matmuls on scratch data while the
            # rows DMAs are in flight get the real matmuls to ~2.4GHz.
            wsrc = consts_pool.tile([128, 128], bf16)
            nc.gpsimd.memset(wsrc[:], 0.0)
            pwarm = psum_pool.tile([128, 512], f32, tag="psum", name="pwarm")
            for _ in range(NWARM):
                nc.tensor.matmul(pwarm[:], wsrc[:], wsrc[:, 0:1].broadcast_to((128, 512)))

            # Warm-up sqrts on ACT: warm1 (scratch input) pulls the Sqrt
            # table load off the critical path; warm2 (consts input) absorbs
            # the consts-DMA wait so the first real sqrt only carries its PE
            # wait (1-embedded-wait walrus limit).
            warm = consts_pool.tile([128, 1], f32)
            nc.scalar.activation(
                warm[:], wsrc[:, 0:1], mybir.ActivationFunctionType.Sqrt,
            )
            warm2 = consts_pool.tile([128, 1], f32)
            nc.scalar.activation(
                warm2[:], consts_tile[:, 0:1], mybir.ActivationFunctionType.Sqrt,
            )

            cols = cols_pool.tile([128, 4 * NC_CH], f32)

            for c in range(NC_CH):
                rc = rows_tiles[c]
                # dummy ldweights absorb this chunk's two ring waits right
                # before first use (q waits are implied: same rings, earlier
                # ticks)
                nc.tensor.ldweights(rc[:, 0, 0:1])
                nc.tensor.ldweights(rc[:, 2, 0:1])
                for bt in range(BT):
                    t = 2 * c + bt
                    bias_ap = consts_tile[:, bt : bt + 1]
                    psum = psum_pool.tile([128, MT], f32, tag="psum",
                                          name=f"ps{c}_{bt}")
                    for h in range(MT // 512):
                        hsl = slice(h * 512, (h + 1) * 512)
                        for dp in range(DCH // 2):
                            lhs = qt_tile[:, 2 * dp : 2 * dp + 2,
                                          bt * 128 : (bt + 1) * 128]
                            rhs = rc[:, 2 * dp : 2 * dp + 2, hsl]
                            nc.tensor.matmul(
                                psum[:, hsl],
                                lhs,
                                rhs,
                                start=(dp == 0),
                                stop=(dp == DCH // 2 - 1),
                                perf_mode=mybir.MatmulPerfMode.DoubleRow,
                            )
                    # sqz = sqrt(bias - psum/128) = d_an/delta, in bf16 so
                    # the DVE epilogue ops hit 4x mode
                    sqz = sqz_pool.tile([128, MT], bf16, tag="sqz",
                                        name=f"sqz{c}_{bt}")
                    nc.scalar.activation(
                        sqz[:], psum[:], mybir.ActivationFunctionType.Sqrt,
                        bias=bias_ap, scale=-2.0 / 256.0,
                    )
                    # C = sum 1[sqz < 1] and S = sum min(sqz, 1), each as a
                    # single 4x-mode tensor_scalar with fused f32 accumulate
                    ind = scr_pool.tile([128, MT], bf16, tag="scr",
                                        name=f"ind{t}")
                    nc.vector.tensor_scalar(
                        ind[:], sqz[:], 1.0, None,
                        op0=mybir.AluOpType.is_lt,
                        op1=mybir.AluOpType.add,
                        accum_out=cols[:, 2 * t : 2 * t + 1],
                    )
                    mn = scr_pool.tile([128, MT], bf16, tag="scr",
                                       name=f"mn{t}")
                    nc.vector.tensor_scalar(
                        mn[:], sqz[:], 1.0, None,
                        op0=mybir.AluOpType.min,
                        op1=mybir.AluOpType.add,
                        accum_out=cols[:, 2 * t + 1 : 2 * t + 2],
                    )

            # single out DMA on the SWDGE ring (fresh completion-sem lane, so
            # no HWDGE lane-recycle wait): all 16 accum writes are DVE ticks,
            # so this carries exactly one wait
            nc.gpsimd.dma_start(out_ext[:], cols[:])

    # Post-pass: the walrus build allows one embedded sync wait per
    # instruction, but Tile emits conservative same-engine self-waits (e.g.
    # a matmul's PE wait when evicting a PSUM slot, or a DVE op's DVE wait
    # when recycling a scratch buffer).  An engine executes its own queue in
    # order, so a wait on the engine's own earlier tick is always implied -
    # drop self-waits whenever another wait remains.
    _eng_prefix = {
        "PE": "PE_", "Activation": "Activation_", "DVE": "DVE_",
        "Pool": "Pool_", "SP": "SP_",
    }
    for bb in nc.m.functions[0].blocks:
        for i in bb.instructions:
            si = i.sync_info
            if si is None:
                continue
            w = si.on_wait
            if len(w) < 2:
                continue
            pref = _eng_prefix.get(getattr(i.engine, "name", None) or str(i.engine))
            if pref is None:
                continue
            keep = [x for x in w if not x.ant_name.startswith(pref)]
            if 1 <= len(keep) < len(w):
                si.on_wait = keep

    return nc


def _get_nc():
    if "nc" not in _cache:
        _cache["nc"] = _build_nc()
    return _cache["nc"]


def _install_ntff_hook():
    """The agent image's antenv lacks axon_hooks; shim it from trn_agent_boot so
    run_bass_kernel_spmd(trace=True) can capture NTFF profiles under axon."""
    import sys
    import types
    try:
        import antenv.axon_hooks  # noqa: F401
        return
    except ImportError:
        pass
    try:
        import antenv
        from trn_agent_boot.trn_boot import _ntff_profile_via_ctypes
        hook = {"h": _ntff_profile_via_ctypes("/opt/axon/libaxon_pjrt.so")}
        mod = types.ModuleType("antenv.axon_hooks")
        mod.get_axon_ntff_profile_hook = lambda: hook["h"]
        mod.set_axon_ntff_profile_hook = lambda h: hook.__setitem__("h", h)
        sys.modules["antenv.axon_hooks"] = mod
        antenv.axon_hooks = mod
    except Exception:
        pass


def kernel(inputs_col, inputs_row, targets_col, targets_row, qidxs, pidxs, nnegs, bs):
    from concourse.bass_utils import run_bass_kernel_spmd

    bs = int(np.asarray(bs))
    assert bs == B and inputs_row.shape == (M, D) and inputs_col.shape[1] == D

    inputs_col = np.asarray(inputs_col, dtype=np.float32)
    inputs_row = np.asarray(inputs_row, dtype=np.float32)
    targets_col = np.asarray(targets_col)
    targets_row = np.asarray(targets_row)
    qidxs = np.asarray(qidxs)
    nnegs = np.asarray(nnegs)

    q = inputs_col[:bs]                                        # [B, D] f32

    # ---- host-side index preprocessing (tiny int ops) ----
    match = targets_col[:bs, None] == qidxs[None, :]
    has_q = match.any(axis=1)
    qloc = match.argmax(axis=1)
    my_nnegs = nnegs[qloc]                                     # [B, K]

    pos_idx = bs + np.arange(bs)
    p = inputs_row[pos_idx]                                    # [B, D] f32

    # ---- per-query constants (f64 host math) ----
    q64 = q.astype(np.float64)
    p64 = p.astype(np.float64)
    na = (q64 * q64).sum(1)
    sa = q64.sum(1)
    # device z = (alpha - 2*sim)/delta^2 with beta_m = |r_m|^2 - 2*eps*sum(r_m)
    # ~= 1 folded in (rows are L2-normalized), so alpha includes the +1.
    alpha = na + 2.0 * EPS * sa + D * EPS * EPS + 1.0
    d_ap = np.sqrt(((q64 - p64 + EPS) ** 2).sum(1))
    gamma = d_ap + TMARGIN
    pos_sim = (q64 * p64).sum(1)
    thr = pos_sim - MARGIN
    delta2 = alpha - 2.0 * thr                 # >= 0.2 (alpha ~ 2, pos_sim <= 1)
    delta = np.where(has_q, np.sqrt(np.maximum(delta2, 1e-12)), 0.0)
    s2 = np.where(has_q, 1.0 / delta2, 0.0)
    bias = np.where(has_q, alpha * s2, 2.0)
    # rows where the masked-sum identity breaks -> exact host fallback
    bad_b = np.flatnonzero(has_q & (delta > gamma))

    # ---- device inputs ----
    # rows{c} per core: [128, DCH, MT], rows[p, k, m] =
    #   inputs_row[core*ML + c*MT + m, k*128 + p] * 16 in fp8
    rt = (inputs_row.T * np.float32(16.0)).astype(ml_dtypes.float8_e4m3)  # [D, M]
    rt = rt.reshape(DCH, 128, NCORES, NC_CH, MT)            # k, p, core, c, m
    qp = (q64 * (16.0 * s2[:, None])).astype(np.float32)
    q_t = qp.T.astype(ml_dtypes.float8_e4m3).reshape(DCH, 128, B)
    q_t = np.ascontiguousarray(q_t.transpose(1, 0, 2))      # [128, DCH, B]
    consts = np.empty((128, 2), np.float32)
    consts[:, 0] = bias[:128]
    consts[:, 1] = bias[128:]

    in_maps = []
    for core in range(NCORES):
        rc = rt[:, :, core].transpose(2, 1, 0, 3)           # [NC_CH, 128, DCH, MT]
        m = {"q_t": q_t, "consts": consts}
        for c in range(NC_CH):
            m[f"rows{c}"] = np.ascontiguousarray(rc[c])
        in_maps.append(m)

    nc = _get_nc()
    trace = bool(os.environ.get("ATHENA_KERNEL_TRACE"))
    if trace:
        _install_ntff_hook()
    r = run_bass_kernel_spmd(nc, in_maps, list(range(NCORES)), trace=trace)
    last_run["exec_time_ns"] = r.exec_time_ns
    last_run["results"] = r

    # ---- gather partials ----
    # cols: col 2t = C of tile t, col 2t+1 = S of tile t  (t = 2*c + bt)
    count_b = np.zeros(B, np.float64)
    smask_b = np.zeros(B, np.float64)   # sum over masked of d_an
    for core in range(NCORES):
        o = np.asarray(r.results[core]["out"], dtype=np.float64)  # [128, 16]
        for c in range(NC_CH):
            for bt in range(BT):
                t = 2 * c + bt
                sl = slice(bt * 128, (bt + 1) * 128)
                C = o[:, 2 * t]
                S = o[:, 2 * t + 1]
                count_b[sl] += C
                # sum_masked d_an = delta * (S - (n - C))
                smask_b[sl] += delta[sl] * (S - (MT - C))
    total_b = gamma * count_b - smask_b

    # ---- exact host fallback for identity violations / non-finite output ----
    bad = set(int(b) for b in bad_b)
    nf = np.flatnonzero(~(np.isfinite(total_b) & np.isfinite(count_b)))
    bad.update(int(b) for b in nf if has_q[b])
    for b in nf:
        if not has_q[b]:
            count_b[b] = 0.0
            total_b[b] = 0.0
    if bad:
        rows64 = inputs_row.astype(np.float64)
        nb_all = (rows64 * rows64).sum(1)
        sb_all = rows64.sum(1)
        for b in sorted(bad):
            simrow = rows64 @ q64[b]
            mask = simrow > thr[b]
            d2 = (na[b] + nb_all - 2.0 * simrow
                  + 2.0 * EPS * (sa[b] - sb_all) + D * EPS * EPS)
            d_an = np.sqrt(np.maximum(d2, 0.0))
            count_b[b] = mask.sum()
            total_b[b] = np.maximum(gamma[b] - d_an, 0.0)[mask].sum()

    # ---- sparse is_nonneg correction (host, exact) ----
    order = np.argsort(targets_row, kind="stable")
    tr_sorted = targets_row[order]
    lo = np.searchsorted(tr_sorted, my_nnegs.ravel(), side="left")
    hi = np.searchsorted(tr_sorted, my_nnegs.ravel(), side="right")
    pairs = set()
    for flat, (l, h) in enumerate(zip(lo, hi)):
        if h > l:
            b = flat // K
            if has_q[b]:
                for mm_ in order[l:h]:
                    pairs.add((b, int(mm_)))
    if pairs:
        pb = np.fromiter((x[0] for x in pairs), np.int64, len(pairs))
        pm = np.fromiter((x[1] for x in pairs), np.int64, len(pairs))
        rows_sel = inputs_row[pm].astype(np.float64)
        sims = (q64[pb] * rows_sel).sum(1)
        sel = sims > thr[pb]
        pb, pm, sims, rows_sel = pb[sel], pm[sel], sims[sel], rows_sel[sel]
        nb = (rows_sel * rows_sel).sum(1)
        sb = rows_sel.sum(1)
        d2 = na[pb] + nb - 2.0 * sims + 2.0 * EPS * (sa[pb] - sb) + D * EPS * EPS
        d_an = np.sqrt(np.maximum(d2, 0.0))
        tl = np.maximum(gamma[pb] - d_an, 0.0)
        np.add.at(count_b, pb, -1.0)
        np.add.at(total_b, pb, -tl)

    neg_count = count_b.sum()
    total = total_b.sum()
    loss = total / neg_count if neg_count > 0 else 0.0
    return np.float32(loss)


# revision 20
# speedup vs baseline: 1.2000x; 1.0947x over previous
"""AdaXbmTripletLoss kernel for 8 Trainium2 NeuronCores (Bass/Tile).

Reference math: loss = sum(hard * relu(d_ap + sqrt(margin) - d_an)) / count(hard)
with hard = ~is_nonneg & (sim > pos_sim - margin) & has_q, over [B=256, M=32768].

Device strategy (inputs_row sharded 8 ways -> ML=4096 rows/core; queries
replicated; all [B, M] work on-device in m-tiles of MT=1024):

z-space trick: host scales each query by 1/delta_b^2 (delta_b = the
d_an threshold sqrt(alpha - 2*thr)), so on device
    z = bias_b - psum/128 = d_an^2 / delta_b^2
and the mask compare becomes the GLOBAL constant 1.0:
    masked  <=>  z < 1  <=>  sqrt(z) < 1.
That removes every per-partition operand from the DVE epilogue ops, so
tensor_scalar runs with immediate scalars on bf16 data -> 4x DVE mode.

Per (c, bt) tile [128 queries x 1024 m]:
  PE:  4 matmuls fp8 DoubleRow -> psum f32 (= 256*sim/delta^2)
  ACT: sqz = Sqrt(-psum/128 + bias_b)  -> bf16 SBUF  [drains PSUM]
  DVE: tensor_scalar min 1.0 + fused f32 accumulate -> S  (all tiles)
  C:   count, split for engine balance (the fused-accumulate DVE op runs at
       1x, ~1.2us/tile, and ACT is 1 elem/cycle too, so the 16 reductions
       are load-balanced across both engines):
         tiles 0..NSIGN-1:  DVE tensor_scalar is_lt 1.0 + accumulate
         tiles NSIGN..7:    ACT Sign(1 - sqz) + accumulator, C = (acc+n)/2
Host per tile: smask += delta*(S - (n - C)); total_b = gamma*count_b - smask_b.
Identity is exact per element for whatever rounding the device applied
(C and S come from the same bf16 sqz values).

DMA: rows chunks alternate DCH-halves on the SP and ACT HWDGE rings in
consumption order, with q's halves FIRST on each ring so the queries land
before the first rows chunk (they previously rode the slow SWDGE ring and
stalled PE by ~4us).  Dummy ldweights absorb the DMA waits
(1-embedded-wait walrus limit); PE clock pre-warmed with dummy matmuls
during the DMA window.  The [128,16] partials return via a single SP-ring
DMA carrying one DVE wait.

Host (numpy, microseconds): index preprocessing, per-query constants in
f64, reduction of the [128, 16] per-core outputs, the sparse is_nonneg
correction (~900 of 8.4M pairs, exact f64), and exact fallbacks for
delta > gamma rows or non-finite device output (never trigger here).
"""

import os
import numpy as np
import ml_dtypes

B = 256
NCOL = 512
M = 32768
D = 512
K = 10
MARGIN = 0.1
EPS = 1e-6
TMARGIN = MARGIN ** 0.5
NCORES = 8
ML = M // NCORES          # 4096 rows per core
DCH = D // 128            # 4 contraction chunks
BT = B // 128             # 2 b-tiles
MT = 1024                 # m-tile size == DMA chunk granularity
NC_CH = ML // MT          # 4 chunks per core
NWARM = 7                 # dummy matmuls to ramp the PE clock
NDVE_C = 4                # tiles whose count runs on DVE; the rest on ACT Sign

_cache = {}
last_run = {}             # exec_time_ns etc. for test harness introspection


def _patch_tile_drain():
    """This container's walrus build allows only ONE embedded sync wait per
    instruction, but TileContext's kernel-tail drain aggregates a wait per
    logical proc (engines + DMA queues) onto a single Drain instruction ->
    'Too many sync wait commands'.  Replace it with standalone single-wait
    wait_ge instructions on the sync engine followed by a bare drain."""
    import concourse.tile as tile
    from concourse.tile_sem_assignment import tick_to_sem

    if getattr(tile.TileContext, "_drain_patched", False):
        return

    def _drain_and_barrier(self, tick_clock, wait_clock):
        gc = tick_clock.global_clock
        assert self.sems is not None
        for proc_idx, sem in sorted(self.sems.allocated().items()):
            tick = gc[proc_idx]
            if tick > 0:
                self.nc.sync.wait_ge(sem, tick_to_sem(tick, proc_idx))
        self.nc.sync.drain()
        self.nc.all_engine_barrier()
        popped = self.nc._tile_sem_poison_stack.pop()
        assert popped is self._sem_poison
        self.nc.clear_and_free_semaphores(list(self.sems.allocated().values()))
        self.nc.all_engine_barrier()

    tile.TileContext._drain_and_barrier = _drain_and_barrier
    tile.TileContext._drain_patched = True


def _build_nc():
    import concourse.bass as bass
    import concourse.mybir as mybir
    import concourse.tile as tile

    _patch_tile_drain()
    nc = bass.Bass()
    f32 = mybir.dt.float32
    bf16 = mybir.dt.bfloat16
    fp8 = mybir.dt.float8e4

    # rows: per chunk [128, DCH, MT] fp8; each chunk is DMA'd as two
    # DCH-halves so the SP and ACT HWDGE rings stream it in parallel, and the
    # ring queues are ordered by PE consumption order (the SDMA engines
    # round-robin the queues, so queue order == global delivery order).
    rows_ext = [nc.declare_dram_parameter(f"rows{c}", [128, DCH, MT], fp8, False)
                for c in range(NC_CH)]
    q_ext = nc.declare_dram_parameter("q_t", [128, DCH, B], fp8, False)
    # consts columns: bias (= alpha/delta^2) for bt0, bt1; ones for Sign bias
    consts_ext = nc.declare_dram_parameter("consts", [128, 4], f32, False)
    # out: col t = S of tile t (DVE); col 8+t = count value of tile t
    # (DVE is_lt count for t < NDVE_C, ACT Sign accumulator otherwise)
    out_ext = nc.declare_dram_parameter("out", [128, 4 * NC_CH], f32, True)

    with tile.TileContext(nc) as tc:
        with (
            tc.tile_pool(name="rows", bufs=1) as rows_pool,
            tc.tile_pool(name="qt", bufs=1) as qt_pool,
            tc.tile_pool(name="consts", bufs=1) as consts_pool,
            tc.tile_pool(name="psum", bufs=4, space="PSUM") as psum_pool,
            tc.tile_pool(name="sqz", bufs=8) as sqz_pool,
            tc.tile_pool(name="scr", bufs=2) as scr_pool,
            tc.tile_pool(name="sgr", bufs=2) as sgr_pool,
            tc.tile_pool(name="cols", bufs=1) as cols_pool,
        ):
            rows_tiles = [rows_pool.tile([128, DCH, MT], fp8, tag=f"rows{c}",
                                         name=f"rows{c}") for c in range(NC_CH)]
            qt_tile = qt_pool.tile([128, DCH, B], fp8)
            consts_tile = consts_pool.tile([128, 4], f32)

            # DMA plan: SP ring gets the low-DCH half of q then every rows
            # chunk, ACT ring the high-DCH halves, both in consumption order
            # (the SDMA engines round-robin the queues, so queue order ==
            # global delivery order); q first so the queries land before the
            # first rows chunk.  consts ride the SP ring last (only needed
            # by the first real sqrt, well after they arrive).
            nc.sync.dma_start(qt_tile[:, 0:2], q_ext[:, 0:2])
            nc.scalar.dma_start(qt_tile[:, 2:4], q_ext[:, 2:4])
            for c in range(NC_CH):
                nc.sync.dma_start(rows_tiles[c][:, 0:2], rows_ext[c][:, 0:2])
            for c in range(NC_CH):
                nc.scalar.dma_start(rows_tiles[c][:, 2:4], rows_ext[c][:, 2:4])
            # consts on the gpsimd SWDGE ring: keeps the HWDGE completion-sem
            # lane recycling down to benign rows-on-q waits (tiny transfer,
            # only needed by the first real sqrt)
            nc.gpsimd.dma_start(consts_tile[:], consts_ext[:])

            # PE clock warm-up: HAM runs the PE at low clock until ~3us of
            # sustained activity.  Dummy matmuls on scratch data while the
            # rows DMAs are in flight get the real matmuls to ~2.4GHz.  The
            # memset rides DVE (idle until the epilogue; gpsimd would issue
            # its consts DMA first and delay the first dummy by ~2.5us).
            wsrc = consts_pool.tile([128, 128], bf16)
            nc.vector.memset(wsrc[:], 0.0)
            pwarm = psum_pool.tile([128, 512], f32, tag="psum", name="pwarm")
            for _ in range(NWARM):
                nc.tensor.matmul(pwarm[:], wsrc[:], wsrc[:, 0:1].broadcast_to((128, 512)))

            # Warm-up sqrts on ACT: warm1 (scratch input) pulls the Sqrt
            # table load off the critical path; warm2 (consts input) absorbs
            # the consts-DMA wait so the first real sqrt only carries its PE
            # wait (1-embedded-wait walrus limit).
            warm = consts_pool.tile([128, 1], f32)
            nc.scalar.activation(
                warm[:], wsrc[:, 0:1], mybir.ActivationFunctionType.Sqrt,
            )
            warm2 = consts_pool.tile([128, 1], f32)
            nc.scalar.activation(
                warm2[:], consts_tile[:, 0:1], mybir.ActivationFunctionType.Sqrt,
            )

            cols = cols_pool.tile([128, 4 * NC_CH], f32)
            ones_ap = consts_tile[:, 2:3]

            for c in range(NC_CH):
                rc = rows_tiles[c]
                # dummy ldweights absorb this chunk's two ring waits right
                # before first use (q waits are implied: same rings, earlier
                # ticks)
                nc.tensor.ldweights(rc[:, 0, 0:1])
                nc.tensor.ldweights(rc[:, 2, 0:1])
                for bt in range(BT):
                    t = 2 * c + bt
                    bias_ap = consts_tile[:, bt : bt + 1]
                    psum = psum_pool.tile([128, MT], f32, tag="psum",
                                          name=f"ps{c}_{bt}")
                    for h in range(MT // 512):
                        hsl = slice(h * 512, (h + 1) * 512)
                        for dp in range(DCH // 2):
                            lhs = qt_tile[:, 2 * dp : 2 * dp + 2,
                                          bt * 128 : (bt + 1) * 128]
                            rhs = rc[:, 2 * dp : 2 * dp + 2, hsl]
                            nc.tensor.matmul(
                                psum[:, hsl],
                                lhs,
                                rhs,
                                start=(dp == 0),
                                stop=(dp == DCH // 2 - 1),
                                perf_mode=mybir.MatmulPerfMode.DoubleRow,
                            )
                    # sqz = sqrt(bias - psum/128) = d_an/delta, in bf16 so
                    # the DVE epilogue ops hit 4x mode
                    sqz = sqz_pool.tile([128, MT], bf16, tag="sqz",
                                        name=f"sqz{c}_{bt}")
                    nc.scalar.activation(
                        sqz[:], psum[:], mybir.ActivationFunctionType.Sqrt,
                        bias=bias_ap, scale=-2.0 / 256.0,
                    )
                    # S = sum min(sqz, 1): tensor_scalar with fused f32
                    # accumulate (1x CACHE_REDUCE on this walrus build)
                    mn = scr_pool.tile([128, MT], bf16, tag="scr",
                                       name=f"mn{t}")
                    nc.vector.tensor_scalar(
                        mn[:], sqz[:], 1.0, None,
                        op0=mybir.AluOpType.min,
                        op1=mybir.AluOpType.add,
                        accum_out=cols[:, t : t + 1],
                    )
                    # C = sum 1[sqz < 1]: split across DVE and ACT so the 16
                    # per-tile reductions load-balance the two engines
                    if t < NDVE_C:
                        ind = scr_pool.tile([128, MT], bf16, tag="scr",
                                            name=f"ind{t}")
                        nc.vector.tensor_scalar(
                            ind[:], sqz[:], 1.0, None,
                            op0=mybir.AluOpType.is_lt,
                            op1=mybir.AluOpType.add,
                            accum_out=cols[:, 8 + t : 9 + t],
                        )
                    else:
                        sg = sgr_pool.tile([128, MT], bf16, tag="sgr",
                                           name=f"sg{t}")
                        nc.scalar.activation(
                            sg[:], sqz[:], mybir.ActivationFunctionType.Sign,
                            bias=ones_ap, scale=-1.0,
                            accum_out=cols[:, 8 + t : 9 + t],
                        )

            # out DMAs on the SWDGE ring (fresh completion-sem lanes, so no
            # HWDGE lane-recycle wait), split by writer engine so each
            # carries exactly one wait (DVE / ACT)
            nc.gpsimd.dma_start(out_ext[:, 0 : 8 + NDVE_C], cols[:, 0 : 8 + NDVE_C])
            nc.gpsimd.dma_start(out_ext[:, 8 + NDVE_C : 16], cols[:, 8 + NDVE_C : 16])

    # Post-pass: the walrus build allows one embedded sync wait per
    # instruction, but Tile emits conservative same-engine self-waits (e.g.
    # a matmul's PE wait when evicting a PSUM slot, or a DVE op's DVE wait
    # when recycling a scratch buffer).  An engine executes its own queue in
    # order, so a wait on the engine's own earlier tick is always implied -
    # drop self-waits whenever another wait remains.
    _eng_prefix = {
        "PE": "PE_", "Activation": "Activation_", "DVE": "DVE_",
        "Pool": "Pool_", "SP": "SP_",
    }
    for bb in nc.m.functions[0].blocks:
        for i in bb.instructions:
            si = i.sync_info
            if si is None:
                continue
            w = si.on_wait
            if len(w) < 2:
                continue
            pref = _eng_prefix.get(getattr(i.engine, "name", None) or str(i.engine))
            if pref is None:
                continue
            keep = [x for x in w if not x.ant_name.startswith(pref)]
            if 1 <= len(keep) < len(w):
                si.on_wait = keep

    return nc


def _get_nc():
    if "nc" not in _cache:
        _cache["nc"] = _build_nc()
    return _cache["nc"]


def _install_ntff_hook():
    """The agent image's antenv lacks axon_hooks; shim it from trn_agent_boot so
    run_bass_kernel_spmd(trace=True) can capture NTFF profiles under axon."""
    import sys
    import types
    try:
        import antenv.axon_hooks  # noqa: F401
        return
    except ImportError:
        pass
    try:
        import antenv
        from trn_agent_boot.trn_boot import _ntff_profile_via_ctypes
        hook = {"h": _ntff_profile_via_ctypes("/opt/axon/libaxon_pjrt.so")}
        mod = types.ModuleType("antenv.axon_hooks")
        mod.get_axon_ntff_profile_hook = lambda: hook["h"]
        mod.set_axon_ntff_profile_hook = lambda h: hook.__setitem__("h", h)
        sys.modules["antenv.axon_hooks"] = mod
        antenv.axon_hooks = mod
    except Exception:
        pass


def kernel(inputs_col, inputs_row, targets_col, targets_row, qidxs, pidxs, nnegs, bs):
    from concourse.bass_utils import run_bass_kernel_spmd

    bs = int(np.asarray(bs))
    assert bs == B and inputs_row.shape == (M, D) and inputs_col.shape[1] == D

    inputs_col = np.asarray(inputs_col, dtype=np.float32)
    inputs_row = np.asarray(inputs_row, dtype=np.float32)
    targets_col = np.asarray(targets_col)
    targets_row = np.asarray(targets_row)
    qidxs = np.asarray(qidxs)
    nnegs = np.asarray(nnegs)

    q = inputs_col[:bs]                                        # [B, D] f32

    # ---- host-side index preprocessing (tiny int ops) ----
    match = targets_col[:bs, None] == qidxs[None, :]
    has_q = match.any(axis=1)
    qloc = match.argmax(axis=1)
    my_nnegs = nnegs[qloc]                                     # [B, K]

    pos_idx = bs + np.arange(bs)
    p = inputs_row[pos_idx]                                    # [B, D] f32

    # ---- per-query constants (f64 host math) ----
    q64 = q.astype(np.float64)
    p64 = p.astype(np.float64)
    na = (q64 * q64).sum(1)
    sa = q64.sum(1)
    # device z = (alpha - 2*sim)/delta^2 with beta_m = |r_m|^2 - 2*eps*sum(r_m)
    # ~= 1 folded in (rows are L2-normalized), so alpha includes the +1.
    alpha = na + 2.0 * EPS * sa + D * EPS * EPS + 1.0
    d_ap = np.sqrt(((q64 - p64 + EPS) ** 2).sum(1))
    gamma = d_ap + TMARGIN
    pos_sim = (q64 * p64).sum(1)
    thr = pos_sim - MARGIN
    delta2 = alpha - 2.0 * thr                 # >= 0.2 (alpha ~ 2, pos_sim <= 1)
    delta = np.where(has_q, np.sqrt(np.maximum(delta2, 1e-12)), 0.0)
    s2 = np.where(has_q, 1.0 / delta2, 0.0)
    bias = np.where(has_q, alpha * s2, 2.0)
    # rows where the masked-sum identity breaks -> exact host fallback
    bad_b = np.flatnonzero(has_q & (delta > gamma))

    # ---- device inputs ----
    # rows{c} per core: [128, DCH, MT], rows[p, k, m] =
    #   inputs_row[core*ML + c*MT + m, k*128 + p] * 16 in fp8
    rt = (inputs_row.T * np.float32(16.0)).astype(ml_dtypes.float8_e4m3)  # [D, M]
    rt = rt.reshape(DCH, 128, NCORES, NC_CH, MT)            # k, p, core, c, m
    qp = (q64 * (16.0 * s2[:, None])).astype(np.float32)
    q_t = qp.T.astype(ml_dtypes.float8_e4m3).reshape(DCH, 128, B)
    q_t = np.ascontiguousarray(q_t.transpose(1, 0, 2))      # [128, DCH, B]
    consts = np.empty((128, 4), np.float32)
    consts[:, 0] = bias[:128]
    consts[:, 1] = bias[128:]
    consts[:, 2] = 1.0
    consts[:, 3] = 0.0

    in_maps = []
    for core in range(NCORES):
        rc = rt[:, :, core].transpose(2, 1, 0, 3)           # [NC_CH, 128, DCH, MT]
        m = {"q_t": q_t, "consts": consts}
        for c in range(NC_CH):
            m[f"rows{c}"] = np.ascontiguousarray(rc[c])
        in_maps.append(m)

    nc = _get_nc()
    trace = bool(os.environ.get("ATHENA_KERNEL_TRACE"))
    if trace:
        _install_ntff_hook()
    r = run_bass_kernel_spmd(nc, in_maps, list(range(NCORES)), trace=trace)
    last_run["exec_time_ns"] = r.exec_time_ns
    last_run["results"] = r

    # ---- gather partials ----
    # cols: col t = S of tile t; col 8+t = C (DVE is_lt) for t < NDVE_C,
    # else the ACT Sign accumulator (C = (acc + n)/2)
    count_b = np.zeros(B, np.float64)
    smask_b = np.zeros(B, np.float64)   # sum over masked of d_an
    for core in range(NCORES):
        o = np.asarray(r.results[core]["out"], dtype=np.float64)  # [128, 16]
        for c in range(NC_CH):
            for bt in range(BT):
                t = 2 * c + bt
                sl = slice(bt * 128, (bt + 1) * 128)
                S = o[:, t]
                if t < NDVE_C:
                    C = o[:, 8 + t]
                else:
                    C = (o[:, 8 + t] + MT) / 2.0
                count_b[sl] += C
                # sum_masked d_an = delta * (S - (n - C))
                smask_b[sl] += delta[sl] * (S - (MT - C))
    total_b = gamma * count_b - smask_b

    # ---- exact host fallback for identity violations / non-finite output ----
    bad = set(int(b) for b in bad_b)
    nf = np.flatnonzero(~(np.isfinite(total_b) & np.isfinite(count_b)))
    bad.update(int(b) for b in nf if has_q[b])
    for b in nf:
        if not has_q[b]:
            count_b[b] = 0.0
            total_b[b] = 0.0
    if bad:
        rows64 = inputs_row.astype(np.float64)
        nb_all = (rows64 * rows64).sum(1)
        sb_all = rows64.sum(1)
        for b in sorted(bad):
            simrow = rows64 @ q64[b]
            mask = simrow > thr[b]
            d2 = (na[b] + nb_all - 2.0 * simrow
                  + 2.0 * EPS * (sa[b] - sb_all) + D * EPS * EPS)
            d_an = np.sqrt(np.maximum(d2, 0.0))
            count_b[b] = mask.sum()
            total_b[b] = np.maximum(gamma[b] - d_an, 0.0)[mask].sum()

    # ---- sparse is_nonneg correction (host, exact) ----
    order = np.argsort(targets_row, kind="stable")
    tr_sorted = targets_row[order]
    lo = np.searchsorted(tr_sorted, my_nnegs.ravel(), side="left")
    hi = np.searchsorted(tr_sorted, my_nnegs.ravel(), side="right")
    pairs = set()
    for flat, (l, h) in enumerate(zip(lo, hi)):
        if h > l:
            b = flat // K
            if has_q[b]:
                for mm_ in order[l:h]:
                    pairs.add((b, int(mm_)))
    if pairs:
        pb = np.fromiter((x[0] for x in pairs), np.int64, len(pairs))
        pm = np.fromiter((x[1] for x in pairs), np.int64, len(pairs))
        rows_sel = inputs_row[pm].astype(np.float64)
        sims = (q64[pb] * rows_sel).sum(1)
        sel = sims > thr[pb]
        pb, pm, sims, rows_sel = pb[sel], pm[sel], sims[sel], rows_sel[sel]
        nb = (rows_sel * rows_sel).sum(1)
        sb = rows_sel.sum(1)
        d2 = na[pb] + nb - 2.0 * sims + 2.0 * EPS * (sa[pb] - sb) + D * EPS * EPS
        d_an = np.sqrt(np.maximum(d2, 0.0))
        tl = np.maximum(gamma[pb] - d_an, 0.0)
        np.add.at(count_b, pb, -1.0)
        np.add.at(total_b, pb, -tl)

    neg_count = count_b.sum()
    total = total_b.sum()
    loss = total / neg_count if neg_count > 0 else 0.0
    return np.float32(loss)


# revision 28
# speedup vs baseline: 1.2507x; 1.0423x over previous
"""AdaXbmTripletLoss kernel for 8 Trainium2 NeuronCores (Bass/Tile).

Reference math: loss = sum(hard * relu(d_ap + sqrt(margin) - d_an)) / count(hard)
with hard = ~is_nonneg & (sim > pos_sim - margin) & has_q, over [B=256, M=32768].

Device strategy (inputs_row sharded 8 ways -> ML=4096 rows/core; queries
replicated; all [B, M] work on-device in m-tiles of MT=1024):

z-space trick: host scales each query by 1/delta_b^2 (delta_b = the
d_an threshold sqrt(alpha - 2*thr)), so on device
    z = bias_b - psum/128 = d_an^2 / delta_b^2
and the mask compare becomes the GLOBAL constant 1.0:
    masked  <=>  z < 1  <=>  sqrt(z) < 1.
That removes every per-partition operand from the DVE epilogue ops, so
tensor_scalar runs with immediate scalars on bf16 data -> 4x DVE mode.

Per (c, bt) tile [128 queries x 1024 m]:
  PE:  4 matmuls fp8 DoubleRow -> psum f32 (= 256*sim/delta^2)
  ACT: sqz = Sqrt(-psum/128 + bias_b)  -> bf16 SBUF  [drains PSUM]
  DVE: tensor_scalar min 1.0 + fused f32 accumulate -> S  (all tiles)
  C:   count, split for engine balance (the fused-accumulate DVE op runs at
       1x, ~1.2us/tile, and ACT is 1 elem/cycle too, so the 16 reductions
       are load-balanced across both engines):
         tiles 0..NSIGN-1:  DVE tensor_scalar is_lt 1.0 + accumulate
         tiles NSIGN..7:    ACT Sign(1 - sqz) + accumulator, C = (acc+n)/2
Host per tile: smask += delta*(S - (n - C)); total_b = gamma*count_b - smask_b.
Identity is exact per element for whatever rounding the device applied
(C and S come from the same bf16 sqz values).

DMA: rows chunks alternate DCH-halves on the SP and ACT HWDGE rings in
consumption order, with q's halves FIRST on each ring so the queries land
before the first rows chunk (they previously rode the slow SWDGE ring and
stalled PE by ~4us).  Dummy ldweights absorb the DMA waits
(1-embedded-wait walrus limit); PE clock pre-warmed with dummy matmuls
during the DMA window.  The [128,16] partials return via a single SP-ring
DMA carrying one DVE wait.

Host (numpy, microseconds): index preprocessing, per-query constants in
f64, reduction of the [128, 16] per-core outputs, the sparse is_nonneg
correction (~900 of 8.4M pairs, exact f64), and exact fallbacks for
delta > gamma rows or non-finite device output (never trigger here).
"""

import os
import numpy as np
import ml_dtypes

B = 256
NCOL = 512
M = 32768
D = 512
K = 10
MARGIN = 0.1
EPS = 1e-6
TMARGIN = MARGIN ** 0.5
NCORES = 8
ML = M // NCORES          # 4096 rows per core
DCH = D // 128            # 4 contraction chunks
BT = B // 128             # 2 b-tiles
MT = 1024                 # m-tile size == DMA chunk granularity
NC_CH = ML // MT          # 4 chunks per core
NWARM = 7                 # dummy matmuls to ramp the PE clock
NDVE_C = 4                # tiles whose count runs on DVE; the rest on ACT Sign

_cache = {}
last_run = {}             # exec_time_ns etc. for test harness introspection


def _patch_tile_drain():
    """This container's walrus build allows only ONE embedded sync wait per
    instruction, but TileContext's kernel-tail drain aggregates a wait per
    logical proc (engines + DMA queues) onto a single Drain instruction ->
    'Too many sync wait commands'.  Replace it with standalone single-wait
    wait_ge instructions on the sync engine followed by a bare drain."""
    import concourse.tile as tile
    from concourse.tile_sem_assignment import tick_to_sem

    if getattr(tile.TileContext, "_drain_patched", False):
        return

    def _drain_and_barrier(self, tick_clock, wait_clock):
        gc = tick_clock.global_clock
        assert self.sems is not None
        for proc_idx, sem in sorted(self.sems.allocated().items()):
            tick = gc[proc_idx]
            if tick > 0:
                self.nc.sync.wait_ge(sem, tick_to_sem(tick, proc_idx))
        self.nc.sync.drain()
        self.nc.all_engine_barrier()
        popped = self.nc._tile_sem_poison_stack.pop()
        assert popped is self._sem_poison
        self.nc.clear_and_free_semaphores(list(self.sems.allocated().values()))
        self.nc.all_engine_barrier()

    tile.TileContext._drain_and_barrier = _drain_and_barrier
    tile.TileContext._drain_patched = True


def _build_nc():
    import concourse.bass as bass
    import concourse.mybir as mybir
    import concourse.tile as tile

    _patch_tile_drain()
    nc = bass.Bass()
    f32 = mybir.dt.float32
    bf16 = mybir.dt.bfloat16
    fp8 = mybir.dt.float8e4

    # rows: per chunk [128, DCH, mlen] fp8; each chunk is DMA'd as two
    # DCH-halves so the SP and ACT HWDGE rings stream it in parallel, and the
    # ring queues are ordered by PE consumption order (the SDMA engines
    # round-robin the queues, so queue order == global delivery order).
    # Chunk 0 is split in two 512-m sub-chunks so the first real matmuls can
    # start ~1.3us earlier — the PE clock ramp must not see a >1us idle gap
    # between the warm-up dummies and the real stream (HAM resets to the
    # mid p-state and the next ~3us of matmuls run at half speed).
    CH_SPLIT = [("rows0a", 512), ("rows0b", 512), ("rows1", MT),
                ("rows2", MT), ("rows3", MT)]
    rows_ext = {name: nc.declare_dram_parameter(name, [128, DCH, mlen], fp8, False)
                for name, mlen in CH_SPLIT}
    q_ext = nc.declare_dram_parameter("q_t", [128, DCH, B], fp8, False)
    # consts columns: bias (= alpha/delta^2) for bt0, bt1; ones for Sign bias
    consts_ext = nc.declare_dram_parameter("consts", [128, 4], f32, False)
    # out: col t = S of tile t (DVE); col 8+t = count value of tile t
    # (DVE is_lt count for t < NDVE_C, ACT Sign accumulator otherwise)
    out_ext = nc.declare_dram_parameter("out", [128, 4 * NC_CH], f32, True)

    with tile.TileContext(nc) as tc:
        with (
            tc.tile_pool(name="rows", bufs=1) as rows_pool,
            tc.tile_pool(name="qt", bufs=1) as qt_pool,
            tc.tile_pool(name="consts", bufs=1) as consts_pool,
            tc.tile_pool(name="psum", bufs=4, space="PSUM") as psum_pool,
            tc.tile_pool(name="sqz", bufs=8) as sqz_pool,
            tc.tile_pool(name="scr", bufs=2) as scr_pool,
            tc.tile_pool(name="sgr", bufs=2) as sgr_pool,
            tc.tile_pool(name="cols", bufs=1) as cols_pool,
        ):
            rows_tiles = {name: rows_pool.tile([128, DCH, mlen], fp8, tag=name,
                                               name=name)
                          for name, mlen in CH_SPLIT}
            qt_tile = qt_pool.tile([128, DCH, B], fp8)
            consts_tile = consts_pool.tile([128, 4], f32)

            # DMA plan: SP ring gets the low-DCH half of q, then consts,
            # then every rows chunk; ACT ring the high-DCH halves, both in
            # consumption order (the SDMA engines round-robin the queues, so
            # queue order == global delivery order).  q first so the queries
            # land before the first rows chunk.  The lo/hi pairs are emitted
            # interleaved so the round-robin HWDGE completion-sem lanes
            # recycle onto long-completed transfers (each lane-recycle wait
            # is then trivially satisfied).
            nc.sync.dma_start(qt_tile[:, 0:2], q_ext[:, 0:2])
            nc.scalar.dma_start(qt_tile[:, 2:4], q_ext[:, 2:4])
            nc.sync.dma_start(consts_tile[:], consts_ext[:])
            for name, _ in CH_SPLIT:
                nc.sync.dma_start(rows_tiles[name][:, 0:2], rows_ext[name][:, 0:2])
                nc.scalar.dma_start(rows_tiles[name][:, 2:4], rows_ext[name][:, 2:4])

            # PE clock warm-up: HAM runs the PE at low clock until ~3us of
            # sustained activity.  Dummy matmuls while the rows DMAs are in
            # flight get the real matmuls to ~2.4GHz.  The source is a raw
            # (non-pool) SBUF tensor read uninitialized, so the first dummy
            # has no producer dependency and starts the moment the PE queue
            # opens; the garbage results land in a discarded psum tile.
            wsrc_t = nc.alloc_sbuf_tensor("wsrc", [128, 128], bf16)
            wsrc = wsrc_t[:, :]
            pwarm = psum_pool.tile([128, 512], f32, tag="psum", name="pwarm")
            for _ in range(NWARM):
                nc.tensor.matmul(pwarm[:], wsrc, wsrc[:, 0:1].broadcast_to((128, 512)))

            # Warm-up sqrts on ACT: warm1 (scratch input, no deps) pulls the
            # Sqrt table load off the critical path; warm2 (consts input)
            # absorbs the consts-DMA wait so the first real sqrt only
            # carries its PE wait (1-embedded-wait walrus limit).
            warm = consts_pool.tile([128, 1], f32)
            nc.scalar.activation(
                warm[:], wsrc_t[:, 0:1], mybir.ActivationFunctionType.Sqrt,
            )
            warm2 = consts_pool.tile([128, 1], f32)
            nc.scalar.activation(
                warm2[:], consts_tile[:, 0:1], mybir.ActivationFunctionType.Sqrt,
            )

            cols = cols_pool.tile([128, 4 * NC_CH], f32)
            ones_ap = consts_tile[:, 2:3]

            # per epilogue chunk c: list of (rows tile, m-slice within tile)
            # for each 512-col h-block
            CH_BLOCKS = {
                0: [(rows_tiles["rows0a"], slice(0, 512)),
                    (rows_tiles["rows0b"], slice(0, 512))],
            }
            for c in range(1, NC_CH):
                CH_BLOCKS[c] = [(rows_tiles[f"rows{c}"], slice(0, 512)),
                                (rows_tiles[f"rows{c}"], slice(512, 1024))]

            for c in range(NC_CH):
                # dummy ldweights absorb the chunk-leading ring waits right
                # before first use, so the first real matmul keeps only its
                # psum-recycle (ACT) wait (q waits are implied: same rings,
                # earlier ticks; the h=1 sub-chunk waits ride the h=1
                # matmuls themselves, which carry no other wait)
                rlead = CH_BLOCKS[c][0][0]
                nc.tensor.ldweights(rlead[:, 0, 0:1])
                nc.tensor.ldweights(rlead[:, 2, 0:1])
                for bt in range(BT):
                    t = 2 * c + bt
                    bias_ap = consts_tile[:, bt : bt + 1]
                    psum = psum_pool.tile([128, MT], f32, tag="psum",
                                          name=f"ps{c}_{bt}")
                    for h, (rtile, msl) in enumerate(CH_BLOCKS[c]):
                        hsl = slice(h * 512, (h + 1) * 512)
                        for dp in range(DCH // 2):
                            lhs = qt_tile[:, 2 * dp : 2 * dp + 2,
                                          bt * 128 : (bt + 1) * 128]
                            rhs = rtile[:, 2 * dp : 2 * dp + 2, msl]
                            nc.tensor.matmul(
                                psum[:, hsl],
                                lhs,
                                rhs,
                                start=(dp == 0),
                                stop=(dp == DCH // 2 - 1),
                                perf_mode=mybir.MatmulPerfMode.DoubleRow,
                            )
                    # sqz = sqrt(bias - psum/128) = d_an/delta, in bf16 so
                    # the DVE epilogue ops hit 4x mode
                    sqz = sqz_pool.tile([128, MT], bf16, tag="sqz",
                                        name=f"sqz{c}_{bt}")
                    nc.scalar.activation(
                        sqz[:], psum[:], mybir.ActivationFunctionType.Sqrt,
                        bias=bias_ap, scale=-2.0 / 256.0,
                    )
                    # S = sum min(sqz, 1): tensor_scalar with fused f32
                    # accumulate (1x CACHE_REDUCE on this walrus build)
                    mn = scr_pool.tile([128, MT], bf16, tag="scr",
                                       name=f"mn{t}")
                    nc.vector.tensor_scalar(
                        mn[:], sqz[:], 1.0, None,
                        op0=mybir.AluOpType.min,
                        op1=mybir.AluOpType.add,
                        accum_out=cols[:, t : t + 1],
                    )
                    # C = sum 1[sqz < 1]: split across DVE and ACT so the 16
                    # per-tile reductions load-balance the two engines
                    if t < NDVE_C:
                        ind = scr_pool.tile([128, MT], bf16, tag="scr",
                                            name=f"ind{t}")
                        nc.vector.tensor_scalar(
                            ind[:], sqz[:], 1.0, None,
                            op0=mybir.AluOpType.is_lt,
                            op1=mybir.AluOpType.add,
                            accum_out=cols[:, 8 + t : 9 + t],
                        )
                    else:
                        sg = sgr_pool.tile([128, MT], bf16, tag="sgr",
                                           name=f"sg{t}")
                        nc.scalar.activation(
                            sg[:], sqz[:], mybir.ActivationFunctionType.Sign,
                            bias=ones_ap, scale=-1.0,
                            accum_out=cols[:, 8 + t : 9 + t],
                        )

            # out DMAs on the SP ring (idle since startup), split by writer
            # engine so each carries one data wait (DVE / ACT).  Each also
            # gets a HWDGE lane-recycle wait on a rows transfer that
            # transitively completed long ago — the post-pass below drops
            # those so the walrus 1-embedded-wait limit holds.
            nc.sync.dma_start(out_ext[:, 0 : 8 + NDVE_C], cols[:, 0 : 8 + NDVE_C])
            nc.sync.dma_start(out_ext[:, 8 + NDVE_C : 16], cols[:, 8 + NDVE_C : 16])

    # Post-pass: the walrus build allows one embedded sync wait per
    # instruction, but Tile emits conservative same-engine self-waits (e.g.
    # a matmul's PE wait when evicting a PSUM slot, or a DVE op's DVE wait
    # when recycling a scratch buffer).  An engine executes its own queue in
    # order, so a wait on the engine's own earlier tick is always implied -
    # drop self-waits whenever another wait remains.
    # Additionally, the two out DMAs carry a HWDGE lane-recycle wait on a
    # rows transfer plus their data wait (DVE/ACT).  The lane's previous
    # occupant is a rows chunk whose completion transitively gates the data
    # wait (rows -> matmul -> sqrt -> accumulate), so the lane wait is
    # provably satisfied — drop it, keep the data wait.
    _eng_prefix = {
        "PE": "PE_", "Activation": "Activation_", "DVE": "DVE_",
        "Pool": "Pool_", "SP": "SP_",
    }
    for bb in nc.m.functions[0].blocks:
        for i in bb.instructions:
            si = i.sync_info
            if si is None:
                continue
            w = si.on_wait
            if len(w) < 2:
                continue
            pref = _eng_prefix.get(getattr(i.engine, "name", None) or str(i.engine))
            if pref is not None:
                keep = [x for x in w if not x.ant_name.startswith(pref)]
                if 1 <= len(keep) < len(w):
                    si.on_wait = keep
                    w = keep
            if len(w) >= 2 and i.opcode == "DMACopy":
                keep = [x for x in w if not x.ant_name.startswith("DMAHW")]
                eng_waits = [x for x in keep
                             if x.ant_name.startswith(("DVE_", "Activation_"))]
                if eng_waits and 1 <= len(keep) < len(w):
                    si.on_wait = keep

    return nc


def _get_nc():
    if "nc" not in _cache:
        _cache["nc"] = _build_nc()
    return _cache["nc"]


def _install_ntff_hook():
    """The agent image's antenv lacks axon_hooks; shim it from trn_agent_boot so
    run_bass_kernel_spmd(trace=True) can capture NTFF profiles under axon."""
    import sys
    import types
    try:
        import antenv.axon_hooks  # noqa: F401
        return
    except ImportError:
        pass
    try:
        import antenv
        from trn_agent_boot.trn_boot import _ntff_profile_via_ctypes
        hook = {"h": _ntff_profile_via_ctypes("/opt/axon/libaxon_pjrt.so")}
        mod = types.ModuleType("antenv.axon_hooks")
        mod.get_axon_ntff_profile_hook = lambda: hook["h"]
        mod.set_axon_ntff_profile_hook = lambda h: hook.__setitem__("h", h)
        sys.modules["antenv.axon_hooks"] = mod
        antenv.axon_hooks = mod
    except Exception:
        pass


def kernel(inputs_col, inputs_row, targets_col, targets_row, qidxs, pidxs, nnegs, bs):
    from concourse.bass_utils import run_bass_kernel_spmd

    bs = int(np.asarray(bs))
    assert bs == B and inputs_row.shape == (M, D) and inputs_col.shape[1] == D

    inputs_col = np.asarray(inputs_col, dtype=np.float32)
    inputs_row = np.asarray(inputs_row, dtype=np.float32)
    targets_col = np.asarray(targets_col)
    targets_row = np.asarray(targets_row)
    qidxs = np.asarray(qidxs)
    nnegs = np.asarray(nnegs)

    q = inputs_col[:bs]                                        # [B, D] f32

    # ---- host-side index preprocessing (tiny int ops) ----
    match = targets_col[:bs, None] == qidxs[None, :]
    has_q = match.any(axis=1)
    qloc = match.argmax(axis=1)
    my_nnegs = nnegs[qloc]                                     # [B, K]

    pos_idx = bs + np.arange(bs)
    p = inputs_row[pos_idx]                                    # [B, D] f32

    # ---- per-query constants (f64 host math) ----
    q64 = q.astype(np.float64)
    p64 = p.astype(np.float64)
    na = (q64 * q64).sum(1)
    sa = q64.sum(1)
    # device z = (alpha - 2*sim)/delta^2 with beta_m = |r_m|^2 - 2*eps*sum(r_m)
    # ~= 1 folded in (rows are L2-normalized), so alpha includes the +1.
    alpha = na + 2.0 * EPS * sa + D * EPS * EPS + 1.0
    d_ap = np.sqrt(((q64 - p64 + EPS) ** 2).sum(1))
    gamma = d_ap + TMARGIN
    pos_sim = (q64 * p64).sum(1)
    thr = pos_sim - MARGIN
    delta2 = alpha - 2.0 * thr                 # >= 0.2 (alpha ~ 2, pos_sim <= 1)
    delta = np.where(has_q, np.sqrt(np.maximum(delta2, 1e-12)), 0.0)
    s2 = np.where(has_q, 1.0 / delta2, 0.0)
    bias = np.where(has_q, alpha * s2, 2.0)
    # rows where the masked-sum identity breaks -> exact host fallback
    bad_b = np.flatnonzero(has_q & (delta > gamma))

    # ---- device inputs ----
    # rows{c} per core: [128, DCH, MT], rows[p, k, m] =
    #   inputs_row[core*ML + c*MT + m, k*128 + p] * 16 in fp8
    rt = (inputs_row.T * np.float32(16.0)).astype(ml_dtypes.float8_e4m3)  # [D, M]
    rt = rt.reshape(DCH, 128, NCORES, NC_CH, MT)            # k, p, core, c, m
    qp = (q64 * (16.0 * s2[:, None])).astype(np.float32)
    q_t = qp.T.astype(ml_dtypes.float8_e4m3).reshape(DCH, 128, B)
    q_t = np.ascontiguousarray(q_t.transpose(1, 0, 2))      # [128, DCH, B]
    consts = np.empty((128, 4), np.float32)
    consts[:, 0] = bias[:128]
    consts[:, 1] = bias[128:]
    consts[:, 2] = 1.0
    consts[:, 3] = 0.0

    in_maps = []
    for core in range(NCORES):
        rc = rt[:, :, core].transpose(2, 1, 0, 3)           # [NC_CH, 128, DCH, MT]
        m = {"q_t": q_t, "consts": consts}
        m["rows0a"] = np.ascontiguousarray(rc[0][:, :, 0:512])
        m["rows0b"] = np.ascontiguousarray(rc[0][:, :, 512:1024])
        for c in range(1, NC_CH):
            m[f"rows{c}"] = np.ascontiguousarray(rc[c])
        in_maps.append(m)

    nc = _get_nc()
    trace = bool(os.environ.get("ATHENA_KERNEL_TRACE"))
    if trace:
        _install_ntff_hook()
    r = run_bass_kernel_spmd(nc, in_maps, list(range(NCORES)), trace=trace)
    last_run["exec_time_ns"] = r.exec_time_ns
    last_run["results"] = r

    # ---- gather partials ----
    # cols: col t = S of tile t; col 8+t = C (DVE is_lt) for t < NDVE_C,
    # else the ACT Sign accumulator (C = (acc + n)/2)
    count_b = np.zeros(B, np.float64)
    smask_b = np.zeros(B, np.float64)   # sum over masked of d_an
    for core in range(NCORES):
        o = np.asarray(r.results[core]["out"], dtype=np.float64)  # [128, 16]
        for c in range(NC_CH):
            for bt in range(BT):
                t = 2 * c + bt
                sl = slice(bt * 128, (bt + 1) * 128)
                S = o[:, t]
                if t < NDVE_C:
                    C = o[:, 8 + t]
                else:
                    C = (o[:, 8 + t] + MT) / 2.0
                count_b[sl] += C
                # sum_masked d_an = delta * (S - (n - C))
                smask_b[sl] += delta[sl] * (S - (MT - C))
    total_b = gamma * count_b - smask_b

    # ---- exact host fallback for identity violations / non-finite output ----
    bad = set(int(b) for b in bad_b)
    nf = np.flatnonzero(~(np.isfinite(total_b) & np.isfinite(count_b)))
    bad.update(int(b) for b in nf if has_q[b])
    for b in nf:
        if not has_q[b]:
            count_b[b] = 0.0
            total_b[b] = 0.0
    if bad:
        rows64 = inputs_row.astype(np.float64)
        nb_all = (rows64 * rows64).sum(1)
        sb_all = rows64.sum(1)
        for b in sorted(bad):
            simrow = rows64 @ q64[b]
            mask = simrow > thr[b]
            d2 = (na[b] + nb_all - 2.0 * simrow
                  + 2.0 * EPS * (sa[b] - sb_all) + D * EPS * EPS)
            d_an = np.sqrt(np.maximum(d2, 0.0))
            count_b[b] = mask.sum()
            total_b[b] = np.maximum(gamma[b] - d_an, 0.0)[mask].sum()

    # ---- sparse is_nonneg correction (host, exact) ----
    order = np.argsort(targets_row, kind="stable")
    tr_sorted = targets_row[order]
    lo = np.searchsorted(tr_sorted, my_nnegs.ravel(), side="left")
    hi = np.searchsorted(tr_sorted, my_nnegs.ravel(), side="right")
    pairs = set()
    for flat, (l, h) in enumerate(zip(lo, hi)):
        if h > l:
            b = flat // K
            if has_q[b]:
                for mm_ in order[l:h]:
                    pairs.add((b, int(mm_)))
    if pairs:
        pb = np.fromiter((x[0] for x in pairs), np.int64, len(pairs))
        pm = np.fromiter((x[1] for x in pairs), np.int64, len(pairs))
        rows_sel = inputs_row[pm].astype(np.float64)
        sims = (q64[pb] * rows_sel).sum(1)
        sel = sims > thr[pb]
        pb, pm, sims, rows_sel = pb[sel], pm[sel], sims[sel], rows_sel[sel]
        nb = (rows_sel * rows_sel).sum(1)
        sb = rows_sel.sum(1)
        d2 = na[pb] + nb - 2.0 * sims + 2.0 * EPS * (sa[pb] - sb) + D * EPS * EPS
        d_an = np.sqrt(np.maximum(d2, 0.0))
        tl = np.maximum(gamma[pb] - d_an, 0.0)
        np.add.at(count_b, pb, -1.0)
        np.add.at(total_b, pb, -tl)

    neg_count = count_b.sum()
    total = total_b.sum()
    loss = total / neg_count if neg_count > 0 else 0.0
    return np.float32(loss)
